# revision 11
# baseline (speedup 1.0000x reference)
"""Cluster-growth step (3x3 wrap stencil + masks + global count) on 8 trn2 cores.

Layout: rows sharded across 8 cores (1024 rows/core). Within a core each of the
128 SBUF partitions owns 8 consecutive rows (+1 halo row above/below), columns
processed in chunks of 1024 (+1 halo byte each side). All stencil taps are then
free-dim offsets: vertical taps are u32-aligned (SWAR over 4 packed u8 cells),
horizontal taps use u8 views at +-1 byte.
"""
import sys
import os

if not any(os.path.isdir(os.path.join(p, "concourse")) for p in sys.path if p):
    sys.path.insert(0, "/opt/trn_rl_repo")

import numpy as np
import concourse.bass as bass
import concourse.bacc as bacc
from concourse import mybir
from concourse.bass_types import AP
from concourse.tile import TileContext
from concourse.bass_utils import run_bass_kernel_spmd

N = 8192            # lattice side
NCORES = 8
ROWS = N // NCORES  # rows per core = 1024
KR = ROWS // 128    # rows per partition = 8
CCH = 1024          # column chunk width
NCH = N // CCH      # chunks per core = 8
UW = N + 32         # u_halo row width in bytes (8192 + 2 halo + 30 pad)
RSTR = CCH + 8      # sbuf row stride for the u tile (1032)

_cache = {}


def _build(thr: float, taps: tuple, reps: int = 1):
    """taps: sorted tuple of (dr, dc) with dr,dc in {-1,0,1}."""
    nc = bacc.Bacc("TRN2", target_bir_lowering=False)
    u_halo = nc.dram_tensor("u_halo", [ROWS + 2, UW], mybir.dt.uint8,
                            kind="ExternalInput")
    dnv = nc.dram_tensor("dnv", [ROWS, N], mybir.dt.uint8, kind="ExternalInput")
    rnd = nc.dram_tensor("rnd", [ROWS, N], mybir.dt.float32, kind="ExternalInput")
    out = nc.dram_tensor("out", [ROWS, N], mybir.dt.uint8, kind="ExternalOutput")
    cnt = nc.dram_tensor("cnt", [128, NCH], mybir.dt.float32, kind="ExternalOutput")

    dnv3 = dnv.rearrange("(p k) c -> p k c", k=KR)
    rnd3 = rnd.rearrange("(p k) c -> p k c", k=KR)
    out3 = out.rearrange("(p k) c -> p k c", k=KR)

    u32 = mybir.dt.uint32
    u8 = mybir.dt.uint8
    f32 = mybir.dt.float32
    OR = mybir.AluOpType.bitwise_or
    AND = mybir.AluOpType.bitwise_and
    XOR = mybir.AluOpType.bitwise_xor

    with TileContext(nc) as tc:
        with tc.tile_pool(name="glob", bufs=1) as gpool:
            tcnt = gpool.tile([128, NCH], f32)
            tones = gpool.tile([128, 1], u32)
            nc.vector.memset(tones[:, :], 0xFFFFFFFF)
            tbias = gpool.tile([128, 1], f32)
            nc.vector.memset(tbias[:, :], float(thr))
            t8 = gpool.tile([128, 1], u32)
            nc.vector.memset(t8[:, :], 8)
            t24 = gpool.tile([128, 1], u32)
            nc.vector.memset(t24[:, :], 24)

            with tc.tile_pool(name="work", bufs=2) as pool, \
                 tc.tile_pool(name="scr", bufs=1) as spool:
                for ch in range(NCH * reps):
                    ch = ch % NCH
                    c0 = ch * CCH
                    tu = pool.tile([128, KR + 2, RSTR], u8, name="tu")
                    # load rows p*KR .. p*KR+KR+1, bytes c0 .. c0+CCH+1
                    src = AP(u_halo, c0, [[KR * UW, 128], [UW, KR + 2], [1, CCH + 2]])
                    nc.gpsimd.dma_start(tu[:, 0:KR + 2, 3:CCH + 5], src)

                    td = pool.tile([128, KR, CCH], u8, name="td")
                    nc.gpsimd.dma_start(td[:, :, :], dnv3[:, :, c0:c0 + CCH])
                    tr = pool.tile([128, KR, CCH], f32, name="tr")
                    nc.gpsimd.dma_start(tr[:, :, :], rnd3[:, :, c0:c0 + CCH])

                    # random accept mask (u8 0/1), strict rand < thr.
                    # Sign(thr - rand) on ACT: +1 -> 1, {0,-1} -> 0 (u8
                    # saturation); exact since fp32 a-b never rounds across 0.
                    acc = spool.tile([128, KR, CCH], u8, name="acc")
                    nc.scalar.activation(
                        acc[:, :, :], tr[:, :, :],
                        mybir.ActivationFunctionType.Sign,
                        bias=tbias[:, :], scale=-1.0,
                    )

                    def uview(dr, dc):
                        # u32 view of the KRxCCH region shifted by (dr, dc); dc must be 0
                        return tu[:, 1 + dr:1 + dr + KR, 4:4 + CCH].bitcast(u32)

                    def u8view(dr, dc):
                        return tu[:, 1 + dr:1 + dr + KR, 4 + dc:4 + dc + CCH]

                    aligned = [t for t in taps if t[1] == 0]
                    misal = [t for t in taps if t[1] != 0]
                    use_shift = os.environ.get("KHSHIFT", "1") == "1"

                    # OR-tree over taps
                    terms = []  # list of (ap_u32,)
                    sc_i = 0

                    def scratch():
                        nonlocal sc_i
                        t = spool.tile([128, KR, CCH], u8, name=f"sc{sc_i % 2}")
                        sc_i += 1
                        return t

                    acc_u32 = None
                    for dr, dc in aligned:
                        v = uview(dr, dc)
                        if acc_u32 is None:
                            acc_u32 = v
                        else:
                            o = scratch()
                            nc.vector.tensor_tensor(o[:, :, :].bitcast(u32),
                                                    acc_u32, v, op=OR)
                            acc_u32 = o[:, :, :].bitcast(u32)
                    if misal and use_shift and acc_u32 is not None:
                        # horizontal taps as u32 shifts chained via STT:
                        # left nbr of lane k = lane k-1 -> (w << 8) | (prev >> 24)
                        # right nbr         = (w >> 8) | (next << 24)
                        LSL = mybir.AluOpType.logical_shift_left
                        LSR = mybir.AluOpType.logical_shift_right
                        for dr, dc in misal:
                            base = tu[:, 1 + dr:1 + dr + KR, 4:4 + CCH].bitcast(u32)
                            if dc == -1:
                                cross = tu[:, 1 + dr:1 + dr + KR, 0:CCH].bitcast(u32)
                                terms = [(base, LSL, t8), (cross, LSR, t24)]
                            else:
                                cross = tu[:, 1 + dr:1 + dr + KR, 8:8 + CCH].bitcast(u32)
                                terms = [(base, LSR, t8), (cross, LSL, t24)]
                            for v, sop, samt in terms:
                                o = scratch()
                                nc.vector.scalar_tensor_tensor(
                                    o[:, :, :].bitcast(u32), v, samt[:, :],
                                    acc_u32, op0=sop, op1=OR)
                                acc_u32 = o[:, :, :].bitcast(u32)
                        nbr = acc_u32
                    else:
                        acc_u8 = None
                        for dr, dc in misal:
                            v = u8view(dr, dc)
                            if acc_u8 is None:
                                acc_u8 = v
                            else:
                                o = scratch()
                                nc.vector.tensor_tensor(o[:, :, :], acc_u8, v, op=OR)
                                acc_u8 = o[:, :, :]
                        if acc_u8 is not None and acc_u32 is not None:
                            o = scratch()
                            nc.vector.tensor_tensor(o[:, :, :].bitcast(u32), acc_u32,
                                                    acc_u8.bitcast(u32), op=OR)
                            nbr = o[:, :, :].bitcast(u32)
                        elif acc_u32 is not None:
                            nbr = acc_u32
                        else:
                            nbr = acc_u8.bitcast(u32)

                    ctr = tu[:, 1:1 + KR, 4:4 + CCH]  # center u8 view

                    # x1 = nbr & ~updated
                    x1 = spool.tile([128, KR, CCH], u8, name="x1")
                    nc.vector.scalar_tensor_tensor(
                        x1[:, :, :].bitcast(u32), ctr.bitcast(u32), tones[:, :],
                        nbr, op0=XOR, op1=AND)
                    # x2 = accept & ~do_not_visit
                    x2 = spool.tile([128, KR, CCH], u8, name="x2")
                    nc.vector.scalar_tensor_tensor(
                        x2[:, :, :].bitcast(u32), td[:, :, :].bitcast(u32),
                        tones[:, :], acc[:, :, :].bitcast(u32), op0=XOR, op1=AND)
                    # new = x1 & x2
                    nw = spool.tile([128, KR, CCH], u8, name="nw")
                    nc.vector.tensor_tensor(nw[:, :, :].bitcast(u32),
                                            x1[:, :, :].bitcast(u32),
                                            x2[:, :, :].bitcast(u32), op=AND)
                    # out = updated | new
                    ow = pool.tile([128, KR, CCH], u8, name="ow")
                    nc.vector.tensor_tensor(ow[:, :, :].bitcast(u32),
                                            ctr.bitcast(u32),
                                            nw[:, :, :].bitcast(u32), op=OR)
                    # per-partition count of new, on ACT (accum_out of a copy)
                    nc.scalar.activation(
                        nw[:, :, :], nw[:, :, :],
                        mybir.ActivationFunctionType.Copy,
                        accum_out=tcnt[:, ch:ch + 1])

                    nc.sync.dma_start(out3[:, :, c0:c0 + CCH], ow[:, :, :])

            nc.sync.dma_start(cnt[:, :], tcnt[:, :])
    nc.finalize()
    return nc


def _get_nc(thr: float, taps: tuple):
    key = (round(float(thr), 9), taps)
    if key not in _cache:
        _cache[key] = _build(thr, taps)
    return _cache[key]


def kernel(do_not_visit, updated, neighbour_kernel, randoms, threshold):
    u = np.ascontiguousarray(updated).view(np.uint8)
    d = np.ascontiguousarray(do_not_visit).view(np.uint8)
    r = np.ascontiguousarray(randoms)
    thr = float(np.asarray(threshold))
    kf = np.flip(np.asarray(neighbour_kernel))
    taps = tuple(sorted(
        (dr - 1, dc - 1)
        for dr in range(3) for dc in range(3) if kf[dr, dc] > 0
    ))

    if not taps:
        un = u.astype(bool)
        return un, np.int32(0), np.bool_(True)

    # build per-core inputs
    uh = np.zeros((N + 2, UW), dtype=np.uint8)
    uh[1:N + 1, 1:N + 1] = u
    uh[0, 1:N + 1] = u[N - 1]
    uh[N + 1, 1:N + 1] = u[0]
    uh[:, 0] = np.concatenate(([u[N - 1, N - 1]], u[:, N - 1], [u[0, N - 1]]))
    uh[:, N + 1] = np.concatenate(([u[N - 1, 0]], u[:, 0], [u[0, 0]]))

    in_maps = []
    for c in range(NCORES):
        r0 = c * ROWS
        in_maps.append(dict(
            u_halo=np.ascontiguousarray(uh[r0:r0 + ROWS + 2]),
            dnv=d[r0:r0 + ROWS],
            rnd=r[r0:r0 + ROWS],
        ))

    nc = _get_nc(thr, taps)
    res = run_bass_kernel_spmd(nc, in_maps, core_ids=list(range(NCORES)))

    outs = [res.results[c]["out"] for c in range(NCORES)]
    updated_new = np.concatenate(outs, axis=0).astype(bool)
    n_new = int(sum(float(res.results[c]["cnt"].sum()) for c in range(NCORES)))
    return updated_new, np.int32(n_new), np.bool_(n_new == 0)


# revision 15
# speedup vs baseline: 1.8200x; 1.8200x over previous
"""Cluster-growth step (3x3 wrap stencil + masks + global count) on 8 trn2 cores.

Layout: rows sharded across 8 cores (1024 rows/core). Within a core each of the
128 SBUF partitions owns 8 consecutive rows (+1 halo row above/below), columns
processed in chunks of 1024 (+1 halo byte each side). All stencil taps are then
free-dim offsets: vertical taps are u32-aligned (SWAR over 4 packed u8 cells),
horizontal taps use u8 views at +-1 byte.
"""
import sys
import os

if not any(os.path.isdir(os.path.join(p, "concourse")) for p in sys.path if p):
    sys.path.insert(0, "/opt/trn_rl_repo")

import numpy as np
import concourse.bass as bass
import concourse.bacc as bacc
from concourse import mybir
from concourse.bass_types import AP
from concourse.tile import TileContext
from concourse.bass_utils import run_bass_kernel_spmd

N = 8192            # lattice side
NCORES = 8
ROWS = N // NCORES  # rows per core = 1024
KR = ROWS // 128    # rows per partition = 8
CCH = 1024          # column chunk width
NCH = N // CCH      # chunks per core = 8
UW = N + 32         # u_halo row width in bytes (8192 + 2 halo + 30 pad)
RSTR = CCH + 8      # sbuf row stride for the u tile (1032)

_cache = {}


def _build(thr: float, taps: tuple, reps: int = 1):
    """taps: sorted tuple of (dr, dc) with dr,dc in {-1,0,1}."""
    CCH = int(os.environ.get("KCCH", "1024"))
    NCH = N // CCH
    RSTR = CCH + 8
    BUFS = int(os.environ.get("KBUFS", "2"))
    load_eng_name = os.environ.get("KLOADDMA", "gpsimd")
    act_cmp = os.environ.get("KACTCMP", "1") == "1"
    act_cnt = os.environ.get("KACTCNT", "1") == "1"
    probe = os.environ.get("KPROBE", "")
    nswq = int(os.environ.get("KSWQ", "1"))
    nc = bacc.Bacc("TRN2", target_bir_lowering=False, num_swdge_queues=nswq)
    load_eng = getattr(nc, load_eng_name)
    mix = os.environ.get("KDMAMIX", "0") == "1"
    eng_tu = nc.sync if mix else load_eng
    eng_td = nc.scalar if mix else load_eng
    eng_rnd = load_eng
    eng_out = nc.scalar if mix else nc.sync
    u_halo = nc.dram_tensor("u_halo", [ROWS + 2, UW], mybir.dt.uint8,
                            kind="ExternalInput")
    dnv = nc.dram_tensor("dnv", [ROWS, N], mybir.dt.uint8, kind="ExternalInput")
    rnd = nc.dram_tensor("rnd", [ROWS, N], mybir.dt.float32, kind="ExternalInput")
    out = nc.dram_tensor("out", [ROWS, N], mybir.dt.uint8, kind="ExternalOutput")
    cnt = nc.dram_tensor("cnt", [128, NCH], mybir.dt.float32, kind="ExternalOutput")

    dnv3 = dnv.rearrange("(p k) c -> p k c", k=KR)
    rnd3 = rnd.rearrange("(p k) c -> p k c", k=KR)
    out3 = out.rearrange("(p k) c -> p k c", k=KR)

    u32 = mybir.dt.uint32
    u8 = mybir.dt.uint8
    f32 = mybir.dt.float32
    OR = mybir.AluOpType.bitwise_or
    AND = mybir.AluOpType.bitwise_and
    XOR = mybir.AluOpType.bitwise_xor

    with TileContext(nc) as tc:
        with tc.tile_pool(name="glob", bufs=1) as gpool:
            tcnt = gpool.tile([128, NCH], f32)
            tones = gpool.tile([128, 1], u32)
            nc.vector.memset(tones[:, :], 0xFFFFFFFF)
            tbias = gpool.tile([128, 1], f32)
            nc.vector.memset(tbias[:, :], float(thr))
            t8 = gpool.tile([128, 1], u32)
            nc.vector.memset(t8[:, :], 8)
            t24 = gpool.tile([128, 1], u32)
            nc.vector.memset(t24[:, :], 24)

            with tc.tile_pool(name="work", bufs=BUFS) as pool, \
                 tc.tile_pool(name="scr", bufs=1) as spool:
                for ch in range(NCH * reps):
                    ch = ch % NCH
                    c0 = ch * CCH
                    tu = pool.tile([128, KR + 2, RSTR], u8, name="tu")
                    # load rows p*KR .. p*KR+KR+1, bytes c0 .. c0+CCH+1
                    src = AP(u_halo, c0, [[KR * UW, 128], [UW, KR + 2], [1, CCH + 2]])
                    eng_tu.dma_start(tu[:, 0:KR + 2, 3:CCH + 5], src)

                    td = pool.tile([128, KR, CCH], u8, name="td")
                    eng_td.dma_start(td[:, :, :], dnv3[:, :, c0:c0 + CCH])
                    tr = pool.tile([128, KR, CCH], f32, name="tr")
                    if probe != "nornd":
                        eng_rnd.dma_start(tr[:, :, :], rnd3[:, :, c0:c0 + CCH])

                    # random accept mask (u8 0/1), strict rand < thr.
                    # Sign(thr - rand) on ACT: +1 -> 1, {0,-1} -> 0 (u8
                    # saturation); exact since fp32 a-b never rounds across 0.
                    acc = spool.tile([128, KR, CCH], u8, name="acc")
                    if probe == "nornd":
                        nc.vector.memset(acc[:, :, :], 1)
                    elif act_cmp:
                        nc.scalar.activation(
                            acc[:, :, :], tr[:, :, :],
                            mybir.ActivationFunctionType.Sign,
                            bias=tbias[:, :], scale=-1.0,
                        )
                    elif True:
                        nc.vector.tensor_scalar(
                            acc[:, :, :], tr[:, :, :], float(thr), None,
                            op0=mybir.AluOpType.is_lt,
                        )

                    def uview(dr, dc):
                        # u32 view of the KRxCCH region shifted by (dr, dc); dc must be 0
                        return tu[:, 1 + dr:1 + dr + KR, 4:4 + CCH].bitcast(u32)

                    def u8view(dr, dc):
                        return tu[:, 1 + dr:1 + dr + KR, 4 + dc:4 + dc + CCH]

                    aligned = [t for t in taps if t[1] == 0]
                    misal = [t for t in taps if t[1] != 0]
                    use_shift = os.environ.get("KHSHIFT", "1") == "1"

                    # OR-tree over taps
                    terms = []  # list of (ap_u32,)
                    sc_i = 0

                    def scratch():
                        nonlocal sc_i
                        t = spool.tile([128, KR, CCH], u8, name=f"sc{sc_i % 3}")
                        sc_i += 1
                        return t

                    acc_u32 = None
                    for dr, dc in aligned:
                        v = uview(dr, dc)
                        if acc_u32 is None:
                            acc_u32 = v
                        else:
                            o = scratch()
                            nc.vector.tensor_tensor(o[:, :, :].bitcast(u32),
                                                    acc_u32, v, op=OR)
                            acc_u32 = o[:, :, :].bitcast(u32)
                    if misal and use_shift and acc_u32 is not None:
                        # horizontal taps as u32 shifts chained via STT:
                        # left nbr of lane k = lane k-1 -> (w << 8) | (prev >> 24)
                        # right nbr         = (w >> 8) | (next << 24)
                        LSL = mybir.AluOpType.logical_shift_left
                        LSR = mybir.AluOpType.logical_shift_right
                        for dr, dc in misal:
                            base = tu[:, 1 + dr:1 + dr + KR, 4:4 + CCH].bitcast(u32)
                            if dc == -1:
                                cross = tu[:, 1 + dr:1 + dr + KR, 0:CCH].bitcast(u32)
                                terms = [(base, LSL, t8), (cross, LSR, t24)]
                            else:
                                cross = tu[:, 1 + dr:1 + dr + KR, 8:8 + CCH].bitcast(u32)
                                terms = [(base, LSR, t8), (cross, LSL, t24)]
                            for v, sop, samt in terms:
                                o = scratch()
                                nc.vector.scalar_tensor_tensor(
                                    o[:, :, :].bitcast(u32), v, samt[:, :],
                                    acc_u32, op0=sop, op1=OR)
                                acc_u32 = o[:, :, :].bitcast(u32)
                        nbr = acc_u32
                    else:
                        acc_u8 = None
                        for dr, dc in misal:
                            v = u8view(dr, dc)
                            if acc_u8 is None:
                                acc_u8 = v
                            else:
                                o = scratch()
                                nc.vector.tensor_tensor(o[:, :, :], acc_u8, v, op=OR)
                                acc_u8 = o[:, :, :]
                        if acc_u8 is not None and acc_u32 is not None:
                            o = scratch()
                            nc.vector.tensor_tensor(o[:, :, :].bitcast(u32), acc_u32,
                                                    acc_u8.bitcast(u32), op=OR)
                            nbr = o[:, :, :].bitcast(u32)
                        elif acc_u32 is not None:
                            nbr = acc_u32
                        else:
                            nbr = acc_u8.bitcast(u32)

                    ctr = tu[:, 1:1 + KR, 4:4 + CCH]  # center u8 view

                    # x1 = nbr & ~updated
                    x1 = spool.tile([128, KR, CCH], u8, name="x1")
                    nc.vector.scalar_tensor_tensor(
                        x1[:, :, :].bitcast(u32), ctr.bitcast(u32), tones[:, :],
                        nbr, op0=XOR, op1=AND)
                    # x2 = accept & ~do_not_visit
                    x2 = spool.tile([128, KR, CCH], u8, name="x2")
                    nc.vector.scalar_tensor_tensor(
                        x2[:, :, :].bitcast(u32), td[:, :, :].bitcast(u32),
                        tones[:, :], acc[:, :, :].bitcast(u32), op0=XOR, op1=AND)
                    # new = x1 & x2
                    nw = spool.tile([128, KR, CCH], u8, name="nw")
                    nc.vector.tensor_tensor(nw[:, :, :].bitcast(u32),
                                            x1[:, :, :].bitcast(u32),
                                            x2[:, :, :].bitcast(u32), op=AND)
                    # out = updated | new
                    ow = pool.tile([128, KR, CCH], u8, name="ow")
                    if probe != "noow":
                        nc.vector.tensor_tensor(ow[:, :, :].bitcast(u32),
                                                ctr.bitcast(u32),
                                                nw[:, :, :].bitcast(u32), op=OR)
                    else:
                        nc.vector.tensor_copy(ow[:, :, :].bitcast(u32),
                                              nw[:, :, :].bitcast(u32))
                    # per-partition count of new
                    if act_cnt:
                        nc.scalar.activation(
                            nw[:, :, :], nw[:, :, :],
                            mybir.ActivationFunctionType.Copy,
                            accum_out=tcnt[:, ch:ch + 1])
                    else:
                        nc.vector.tensor_reduce(
                            tcnt[:, ch:ch + 1], nw[:, :, :],
                            axis=mybir.AxisListType.XY,
                            op=mybir.AluOpType.add)

                    eng_out.dma_start(out3[:, :, c0:c0 + CCH], ow[:, :, :])

            nc.sync.dma_start(cnt[:, :], tcnt[:, :])
    nc.finalize()
    return nc


def _get_nc(thr: float, taps: tuple):
    key = (round(float(thr), 9), taps)
    if key not in _cache:
        _cache[key] = _build(thr, taps)
    return _cache[key]


def kernel(do_not_visit, updated, neighbour_kernel, randoms, threshold):
    u = np.ascontiguousarray(updated).view(np.uint8)
    d = np.ascontiguousarray(do_not_visit).view(np.uint8)
    r = np.ascontiguousarray(randoms)
    thr = float(np.asarray(threshold))
    kf = np.flip(np.asarray(neighbour_kernel))
    taps = tuple(sorted(
        (dr - 1, dc - 1)
        for dr in range(3) for dc in range(3) if kf[dr, dc] > 0
    ))

    if not taps:
        un = u.astype(bool)
        return un, np.int32(0), np.bool_(True)

    # build per-core inputs
    uh = np.zeros((N + 2, UW), dtype=np.uint8)
    uh[1:N + 1, 1:N + 1] = u
    uh[0, 1:N + 1] = u[N - 1]
    uh[N + 1, 1:N + 1] = u[0]
    uh[:, 0] = np.concatenate(([u[N - 1, N - 1]], u[:, N - 1], [u[0, N - 1]]))
    uh[:, N + 1] = np.concatenate(([u[N - 1, 0]], u[:, 0], [u[0, 0]]))

    in_maps = []
    for c in range(NCORES):
        r0 = c * ROWS
        in_maps.append(dict(
            u_halo=np.ascontiguousarray(uh[r0:r0 + ROWS + 2]),
            dnv=d[r0:r0 + ROWS],
            rnd=r[r0:r0 + ROWS],
        ))

    nc = _get_nc(thr, taps)
    res = run_bass_kernel_spmd(nc, in_maps, core_ids=list(range(NCORES)))

    outs = [res.results[c]["out"] for c in range(NCORES)]
    updated_new = np.concatenate(outs, axis=0).astype(bool)
    n_new = int(sum(float(res.results[c]["cnt"].sum()) for c in range(NCORES)))
    return updated_new, np.int32(n_new), np.bool_(n_new == 0)
